# revision 22
# baseline (speedup 1.0000x reference)
"""Trainium2 Bass kernel for nn_CDTripletLoss (segment_reduce).

Strategy: community-sharded data layout. Host sorts nodes by community,
pads each community to 256 slots, assigns 64 communities per core.
Device computes per-community sums via tensor_scalar accumulators (4x
DVE mode), pre-scales to -2*mean (bf16), appends the local c2 = |mean|^2
as a bf16 payload column, and AllGathers the [64,129] blocks into the
full [512,129] matrix (no 1.875x AllReduce penalty). Sq-sums run during
the collective (split DVE bn_stats / ACT Square); per-node c2 columns
and all phase-D precomputables are built locally pre-collective. Per-
node distances come from bf16 matmuls, sqrt on ScalarE straight to
fp16, row-sums via tensor_scalar f32 accumulators, own-masked min via
an fp16 tensor_tensor min tree, own-community d^2 extracted pre-sqrt
from PSUM via ACT copies. Low-priority filler matmuls keep the PE
p-state high across the collective. Host combines the 8 cores' sums.
"""
import numpy as np
import ml_dtypes

import concourse.bass as bass
import concourse.tile as tile
from concourse import bacc, mybir
from concourse.bass_utils import run_bass_kernel_spmd

f32 = mybir.dt.float32
bf16 = mybir.dt.bfloat16
fp16 = mybir.dt.float16
AX = mybir.AxisListType
OP = mybir.AluOpType
ACTF = mybir.ActivationFunctionType

NCORES = 8
C = 512            # communities
CPC = 64           # communities per core
KSLOT = 256        # padded slots per community
SLOTS = CPC * KSLOT   # 16384 slots per core
D = 128
NT = SLOTS // 128  # 128 node tiles per core
GRP = 4            # tiles per dist group
NGRP = NT // GRP
HB = CPC // 2      # communities whose sq-sums go through DVE bn_stats
N_NODES = 100000
ALPHA = 0.25
BIG16 = 60000.0
A2_FLOOR_MS = 0.019   # keep A2 off DVE until the collective input is built
NWARM = 75
WARM_FLOOR_MS = 0.0235           # PE p-state keep-alive matmuls across the collective

_PROG = None


def _build_program():
    nc = bacc.Bacc("TRN2", target_bir_lowering=False, debug=False, num_devices=NCORES)

    xT_in = nc.declare_dram_parameter("xT", [D, SLOTS], bf16, isOutput=False)
    aug_in = nc.declare_dram_parameter("aug", [3, SLOTS], bf16, isOutput=False)
    onesc_in = nc.declare_dram_parameter("onesc", [2, C], bf16, isOutput=False)
    x2c_in = nc.declare_dram_parameter("x2c", [128, NT], f32, isOutput=False)
    valid_in = nc.declare_dram_parameter("valid", [128, NT], f32, isOutput=False)
    cntc_in = nc.declare_dram_parameter("cntc", [128, NT], f32, isOutput=False)
    acol_in = nc.declare_dram_parameter("acol", [128, NT], f32, isOutput=False)
    cnt2c_in = nc.declare_dram_parameter("cnt2c", [128, NT], f32, isOutput=False)
    cntrow_in = nc.declare_dram_parameter("cntrow", [1, CPC], f32, isOutput=False)
    cm1r_in = nc.declare_dram_parameter("cm1r", [1, CPC], f32, isOutput=False)
    nscl_in = nc.declare_dram_parameter("nscl", [CPC, 1], f32, isOutput=False)
    prot_in = nc.declare_dram_parameter("prot", [C, C], bf16, isOutput=False)
    ident_in = nc.declare_dram_parameter("ident", [128, 128], f32, isOutput=False)

    out_d = nc.declare_dram_parameter("out", [17, 1], f32, isOutput=True)

    with tile.TileContext(nc, num_cores=NCORES) as tc:
        with (
            tc.tile_pool(name="pers", bufs=1) as pers,
            tc.tile_pool(name="dist", bufs=4) as distp,
            tc.tile_pool(name="dram", bufs=1, space="DRAM") as dramp,
        ):
            # ---------- Phase 0a: critical loads on the Pool queue ----------
            xT = pers.tile([D, SLOTS], bf16)
            xchunks = [512, 1536] + [2048] * 7
            off = 0
            for w in xchunks:
                nc.gpsimd.dma_start(xT[:, off : off + w], xT_in[:, off : off + w])
                off += w
            ident = pers.tile([128, 128], f32)
            nc.gpsimd.dma_start(ident[:], ident_in[:])
            nscl = pers.tile([CPC, 1], f32)
            nc.gpsimd.dma_start(nscl[:], nscl_in[:])
            # tiny early loads on the SP queue
            cntrow = pers.tile([1, CPC], f32)
            nc.sync.dma_start(cntrow[:], cntrow_in[:])
            cm1r = pers.tile([1, CPC], f32)
            nc.sync.dma_start(cm1r[:], cm1r_in[:])
            c2x2r = pers.tile([3, C], bf16)
            nc.sync.dma_start(c2x2r[1:3, :], onesc_in[:])

            ones1 = pers.tile([1, 128], f32)
            nc.vector.memset(ones1[:], 1.0)
            onescol = pers.tile([128, 1], f32)
            nc.vector.memset(onescol[:], 1.0)

            # ---------- Phase A1: per-community sums (DVE TSP, 4x mode) ----------
            Ssum = pers.tile([128, CPC], f32)
            ascr = [pers.tile([128, KSLOT], bf16, name=f"ascr{i}") for i in range(4)]
            for j in range(CPC):
                nc.vector.tensor_scalar(ascr[j % 4][:],
                                        xT[:, KSLOT * j : KSLOT * (j + 1)],
                                        1.0, 0.0, op0=OP.mult, op1=OP.add,
                                        accum_out=Ssum[:, j : j + 1])

            with tc.tile_pool(name="psB", bufs=2, space="PSUM") as psB:
                # ---------- Phase B1: transpose+scale, local c2, AllGather ----------
                # collective payload per community j: [-2*mean (128 cols) | c2hi]
                m2loc = pers.tile([CPC, 129], bf16)
                msql = pers.tile([CPC, 128], f32)
                c2loc = pers.tile([CPC, 1], f32)
                ccin = dramp.tile([CPC, 129], bf16)
                HF = CPC // 2
                for hf in range(2):
                    sl = slice(HF * hf, HF * (hf + 1))
                    ps_t = psB.tile([HF, 128], f32, tag="pst")
                    nc.tensor.transpose(ps_t[:], Ssum[:, sl], ident[:])
                    nc.vector.tensor_scalar(m2loc[sl, 0:128], ps_t[:],
                                            nscl[sl, 0:1], None, op0=OP.mult)
                    nc.vector.tensor_mul(msql[sl, :], m2loc[sl, 0:128],
                                         m2loc[sl, 0:128])
                    nc.vector.tensor_reduce(c2loc[sl, :], msql[sl, :],
                                            axis=AX.X, op=OP.add)
                    nc.vector.tensor_scalar(c2loc[sl, :], c2loc[sl, :], 0.25, None,
                                            op0=OP.mult)
                    nc.vector.tensor_copy(m2loc[sl, 128:129], c2loc[sl, :])
                    nc.gpsimd.dma_start(ccin[sl, :], m2loc[sl, :])
                ccout = dramp.tile([C, 129], bf16)
                nc.gpsimd.collective_compute(
                    "AllGather", OP.bypass,
                    replica_groups=[list(range(NCORES))],
                    ins=[ccin[:].opt()],
                    outs=[ccout[:].opt()],
                )

                # ---------- Phase 0b: bulk loads (Pool queue, after ccin) ----------
                aug = pers.tile([3, SLOTS], bf16)
                nc.gpsimd.dma_start(aug[:], aug_in[:])
                prot = pers.tile([128, 4, C], bf16)
                nc.gpsimd.dma_start(prot[:],
                                    prot_in.rearrange("(ch c) f -> c ch f", ch=4))
                x2c = pers.tile([128, NT], f32)
                nc.gpsimd.dma_start(x2c[:], x2c_in[:])
                valid = pers.tile([128, NT], f32)
                nc.gpsimd.dma_start(valid[:], valid_in[:])
                cntc = pers.tile([128, NT], f32)
                nc.gpsimd.dma_start(cntc[:], cntc_in[:])
                acol = pers.tile([128, NT], f32)
                nc.gpsimd.dma_start(acol[:], acol_in[:])
                cnt2c = pers.tile([128, NT], f32)
                nc.gpsimd.dma_start(cnt2c[:], cnt2c_in[:])

                # local c2 rows (SBUF partition->free moves via SP DMA)
                c2row = pers.tile([1, CPC], f32)
                nc.sync.dma_start(c2row[:], c2loc[:, 0:1])
                c2hirow = pers.tile([1, CPC], bf16)
                nc.sync.dma_start(c2hirow[:], m2loc[:, 128:129])

                # ---------- Phase A2 (overlaps collective) ----------
                SQloc = pers.tile([128, CPC], f32)
                qscr = pers.tile([128, KSLOT], bf16)
                for j in range(HB, CPC):
                    nc.scalar.activation(qscr[:], xT[:, KSLOT * j : KSLOT * (j + 1)],
                                         ACTF.Square, accum_out=SQloc[:, j : j + 1])
                # preload the Sqrt act table off the critical path
                sq1 = pers.tile([1, 128], f32)
                nc.scalar.activation(sq1[:], ones1[:], ACTF.Sqrt)
                # PE p-state keep-alive across the collective (lowest priority)
                with tc.high_priority(offset=-1000000):
                    with tc.tile_wait_until(WARM_FLOOR_MS):
                        ps_w = psB.tile([128, 512], f32, tag="warm")
                        for _ in range(NWARM):
                            nc.tensor.matmul(ps_w[:], xT[:, 0:128], xT[:, 0:512],
                                             start=True, stop=True)
                with tc.tile_wait_until(A2_FLOOR_MS):
                    bnb = pers.tile([128, HB, 6], f32)
                    for j in range(HB):
                        nc.vector.bn_stats(bnb[:, j, :],
                                           xT[:, KSLOT * j : KSLOT * (j + 1)])
                    mA = bnb[:, :, 1]
                    m2A = bnb[:, :, 2]
                    mB = bnb[:, :, 4]
                    m2B = bnb[:, :, 5]
                    tA = pers.tile([128, HB], f32)
                    nc.vector.tensor_mul(tA[:], mA, mA)
                    tB = pers.tile([128, HB], f32)
                    nc.vector.tensor_mul(tB[:], mB, mB)
                    nc.vector.tensor_add(tA[:], tA[:], tB[:])
                    nc.vector.tensor_scalar(tA[:], tA[:], 128.0, None, op0=OP.mult)
                    nc.vector.tensor_add(SQloc[:, 0:HB], m2A, m2B)
                    nc.vector.tensor_add(SQloc[:, 0:HB], SQloc[:, 0:HB], tA[:])

                    # broadcasts + phase-D precomputables (all local)
                    ps_cb = psB.tile([128, CPC], f32, tag="ps")
                    nc.tensor.matmul(ps_cb[:], ones1[:], cntrow[:],
                                     start=True, stop=True)
                    cntbc = pers.tile([128, CPC], f32)
                    nc.vector.tensor_copy(cntbc[:], ps_cb[:])
                    ps_cb2 = psB.tile([128, CPC], f32, tag="ps")
                    nc.tensor.matmul(ps_cb2[:], ones1[:], cm1r[:],
                                     start=True, stop=True)
                    cm1bc = pers.tile([128, CPC], f32)
                    nc.vector.tensor_copy(cm1bc[:], ps_cb2[:])
                    # transpose local msq for the std path: [64,128] -> [128,64]
                    ps_q = psB.tile([128, CPC], f32, tag="ps")
                    nc.tensor.transpose(ps_q[:], msql[:], ident[0:CPC, 0:CPC])
                    msqT = pers.tile([128, CPC], f32)
                    nc.vector.tensor_copy(msqT[:], ps_q[:])
                    # per-node c2 columns (accurate f32 + bf16-consistent)
                    c2rep = pers.tile([1, NT], f32)
                    nc.vector.tensor_copy(c2rep[0:1, 0:NT:2], c2row[0:1, :])
                    nc.vector.tensor_copy(c2rep[0:1, 1:NT:2], c2row[0:1, :])
                    c2repb = pers.tile([1, NT], f32)
                    nc.vector.tensor_copy(c2repb[0:1, 0:NT:2], c2hirow[0:1, :])
                    nc.vector.tensor_copy(c2repb[0:1, 1:NT:2], c2hirow[0:1, :])
                    ps_cc = psB.tile([128, NT], f32, tag="ps")
                    nc.tensor.matmul(ps_cc[:], ones1[:], c2rep[:],
                                     start=True, stop=True)
                    c2cols = pers.tile([128, NT], f32)
                    nc.vector.tensor_copy(c2cols[:], ps_cc[:])
                    ps_cc2 = psB.tile([128, NT], f32, tag="ps")
                    nc.tensor.matmul(ps_cc2[:], ones1[:], c2repb[:],
                                     start=True, stop=True)
                    c2bc = pers.tile([128, NT], f32)
                    nc.vector.tensor_copy(c2bc[:], ps_cc2[:])
                    ap1 = pers.tile([128, NT], f32)
                    nc.vector.tensor_scalar(ap1[:], acol[:], 1.0, None, op0=OP.add)
                    aa = pers.tile([128, NT], f32)
                    nc.vector.tensor_mul(aa[:], acol[:], acol[:])
                    a_ap1 = pers.tile([128, NT], f32)
                    nc.vector.tensor_mul(a_ap1[:], acol[:], ap1[:])
                    t2 = pers.tile([128, NT], f32)
                    nc.vector.tensor_mul(t2[:], ap1[:], ap1[:])
                    nc.vector.tensor_mul(t2[:], t2[:], x2c[:])
                    # xpc = x2 + c2(bf16); nac = -acol*ap1*cnt; posb = aa*S2 + t2
                    xpc = pers.tile([128, NT], f32)
                    nc.vector.tensor_add(xpc[:], x2c[:], c2bc[:])
                    nac = pers.tile([128, NT], f32)
                    nc.vector.tensor_mul(nac[:], a_ap1[:], cntc[:])
                    nc.vector.tensor_scalar(nac[:], nac[:], -1.0, None, op0=OP.mult)
                    S2 = pers.tile([128, NT], f32)
                    nc.vector.tensor_mul(S2[:], c2cols[:], cnt2c[:])
                    posb = pers.tile([128, NT], f32)
                    nc.vector.tensor_mul(posb[:], aa[:], S2[:])
                    nc.vector.tensor_add(posb[:], posb[:], t2[:])
                    # std-path algebra, fully local
                    sq = pers.tile([128, CPC], f32)
                    nc.vector.tensor_mul(sq[:], msqT[:], cntbc[:])
                    nc.vector.tensor_scalar(sq[:], sq[:], -0.25, None, op0=OP.mult)
                    nc.vector.tensor_add(sq[:], sq[:], SQloc[:])
                    nc.vector.tensor_mul(sq[:], sq[:], cm1bc[:])
                    nc.vector.tensor_scalar(sq[:], sq[:], 0.0, None, op0=OP.max)
                    stdv = pers.tile([128, CPC], f32)
                    nc.scalar.activation(stdv[:], sq[:], ACTF.Sqrt)
                    nc.vector.tensor_scalar(stdv[:], stdv[:], -1.0, None, op0=OP.add)
                    nc.vector.tensor_mul(stdv[:], stdv[:], stdv[:])

                # ---------- Phase B2: gather, rotate means + c2 row ----------
                gsb = pers.tile([128, 4, 129], bf16)
                nc.sync.dma_start(gsb[:],
                                  ccout.rearrange("(ch c) d -> c ch d", ch=4))
                ps_r = psB.tile([128, C], f32, tag="ps")
                for ch in range(4):
                    nc.tensor.matmul(ps_r[:], gsb[:, ch, 0:128], prot[:, ch, :],
                                     start=(ch == 0), stop=(ch == 3))
                m2T = pers.tile([128, C], bf16)
                nc.vector.tensor_copy(m2T[:], ps_r[:])
                ps_c2 = psB.tile([1, C], f32, tag="ps")
                for ch in range(4):
                    nc.tensor.matmul(ps_c2[:], gsb[:, ch, 128:129], prot[:, ch, :],
                                     start=(ch == 0), stop=(ch == 3))
                nc.vector.tensor_copy(c2x2r[0:1, :], ps_c2[0:1, :])

            # ---------- Phase C: distance tiles ----------
            H = NT // 4
            sumcq = [pers.tile([128, H], f32, name=f"sumcq{i}") for i in range(4)]
            own2q = [pers.tile([128, H], f32, name=f"own2q{i}") for i in range(4)]
            mincq = [pers.tile([128, H], f32, name=f"mincq{i}") for i in range(4)]
            with tc.tile_pool(name="psC", bufs=2, space="PSUM") as psC:
                for g in range(NGRP):
                    ps4 = psC.tile([128, GRP * C], f32, tag="ps")
                    for q in range(GRP):
                        t = GRP * g + q
                        nc.tensor.matmul(ps4[:, C * q : C * (q + 1)],
                                         xT[:, 128 * t : 128 * (t + 1)],
                                         m2T[:], start=True, stop=False)
                        nc.tensor.matmul(ps4[:, C * q : C * (q + 1)],
                                         aug[:, 128 * t : 128 * (t + 1)],
                                         c2x2r[:], start=False, stop=True)
                    # own cols of tiles 4g..4g+3 sit at stride-C pairs:
                    # [2g, C+2g] and [2C+2g+1, 3C+2g+1]
                    o0 = 2 * g
                    o1 = 2 * C + 2 * g + 1
                    qd, qb = g // 8, 4 * (g % 8)
                    nc.scalar.activation(own2q[qd][:, qb : qb + 2],
                                         ps4[:, o0 : o0 + C + 1 : C], ACTF.Copy)
                    nc.scalar.activation(own2q[qd][:, qb + 2 : qb + 4],
                                         ps4[:, o1 : o1 + C + 1 : C], ACTF.Copy)
                    d16 = distp.tile([128, GRP, C], fp16)
                    d16f = d16.rearrange("p q c -> p (q c)")
                    nc.scalar.activation(d16f[:], ps4[:], ACTF.Sqrt)
                    for q in range(GRP):
                        nc.vector.tensor_scalar(d16[:, q, :], d16[:, q, :], 1.0, 0.0,
                                                op0=OP.mult, op1=OP.add,
                                                accum_out=sumcq[qd][:, qb + q : qb + q + 1])
                    nc.vector.memset(d16f[:, o0 : o0 + C + 1 : C], BIG16)
                    nc.vector.memset(d16f[:, o1 : o1 + C + 1 : C], BIG16)
                    s1 = distp.tile([128, GRP, C // 2], fp16)
                    nc.vector.tensor_tensor(s1[:], d16[:, :, 0 : C // 2],
                                            d16[:, :, C // 2 : C], op=OP.min)
                    s2 = distp.tile([128, GRP, C // 4], fp16)
                    nc.vector.tensor_tensor(s2[:], s1[:, :, 0 : C // 4],
                                            s1[:, :, C // 4 : C // 2], op=OP.min)
                    s3 = distp.tile([128, GRP, C // 8], fp16)
                    nc.vector.tensor_tensor(s3[:], s2[:, :, 0 : C // 8],
                                            s2[:, :, C // 8 : C // 4], op=OP.min)
                    nc.vector.tensor_reduce(mincq[qd][:, qb : qb + 4], s3[:],
                                            axis=AX.X, op=OP.min)

            # ---------- Phase D: per-node algebra (quarters) ----------
            ownc = pers.tile([128, NT], f32)
            tx = pers.tile([128, NT], f32)
            pos2 = pers.tile([128, NT], f32)
            pos = pers.tile([128, NT], f32)
            mneg = pers.tile([128, NT], f32)
            tm = pers.tile([128, NT], f32)
            tn = pers.tile([128, NT], f32)
            red = pers.tile([128, 17], f32)
            HC = NT // 8
            for h in range(8):
                s = slice(HC * h, HC * (h + 1))
                qd = h // 2
                sq_ = slice(HC * (h % 2), HC * (h % 2 + 1))
                nc.scalar.activation(ownc[:, s], own2q[qd][:, sq_], ACTF.Sqrt)
                nc.vector.tensor_sub(tx[:, s], xpc[:, s], own2q[qd][:, sq_])
                nc.vector.tensor_mul(tx[:, s], tx[:, s], nac[:, s])
                nc.vector.tensor_add(pos2[:, s], posb[:, s], tx[:, s])
                nc.vector.tensor_scalar(pos2[:, s], pos2[:, s], 0.0, None,
                                        op0=OP.max)
                nc.scalar.activation(pos[:, s], pos2[:, s], ACTF.Sqrt)
                nc.vector.tensor_sub(mneg[:, s], sumcq[qd][:, sq_], ownc[:, s])
                nc.vector.tensor_scalar(mneg[:, s], mneg[:, s], 1.0 / (C - 1),
                                        None, op0=OP.mult)
                nc.vector.tensor_sub(tm[:, s], pos[:, s], mneg[:, s])
                nc.vector.tensor_scalar(tm[:, s], tm[:, s], ALPHA, 0.0,
                                        op0=OP.add, op1=OP.max)
                nc.vector.tensor_mul(tm[:, s], tm[:, s], valid[:, s])
                nc.vector.tensor_reduce(red[:, h : h + 1], tm[:, s],
                                        axis=AX.X, op=OP.add)
                nc.vector.tensor_sub(tn[:, s], pos[:, s], mincq[qd][:, sq_])
                nc.vector.tensor_scalar(tn[:, s], tn[:, s], ALPHA, 0.0,
                                        op0=OP.add, op1=OP.max)
                nc.vector.tensor_mul(tn[:, s], tn[:, s], valid[:, s])
                nc.vector.tensor_reduce(red[:, 8 + h : 9 + h], tn[:, s],
                                        axis=AX.X, op=OP.add)
            nc.vector.tensor_reduce(red[:, 16:17], stdv[:], axis=AX.X, op=OP.add)
            with tc.tile_pool(name="psD", bufs=1, space="PSUM") as psD:
                ps_f = psD.tile([17, 1], f32, tag="psf")
                nc.tensor.matmul(ps_f[:], red[:], onescol[:], start=True, stop=True)
                outsb = pers.tile([17, 1], f32)
                nc.vector.tensor_copy(outsb[:], ps_f[:])
                nc.sync.dma_start(out_d[:], outsb[:])

    nc.compile()
    return nc


def _host_prep(node_features, community_belong_list):
    x = np.asarray(node_features)
    if x.ndim == 3:
        x = x[0]
    x = np.ascontiguousarray(x, dtype=np.float32)  # [N, D]
    cbl = np.asarray(community_belong_list).astype(np.int64)
    N = x.shape[0]
    comm = np.empty(N, np.int64)
    comm[cbl[0]] = cbl[1]
    counts = np.bincount(comm, minlength=C).astype(np.int64)
    assert counts.min() >= 2, f"community with <2 nodes: {counts.min()}"
    assert counts.max() <= KSLOT, f"community overflow: {counts.max()} > {KSLOT}"

    order = np.argsort(comm, kind="stable")
    comm_sorted = comm[order]
    cstart = np.zeros(C + 1, np.int64)
    np.cumsum(counts, out=cstart[1:])
    ranks = np.arange(N) - cstart[comm_sorted]
    slot_global = comm_sorted * KSLOT + ranks

    X_pad = np.zeros((C * KSLOT, D), np.float32)
    X_pad[slot_global] = x[order]
    x2 = (x.astype(np.float64) ** 2).sum(1).astype(np.float32)
    x2_pad = np.zeros(C * KSLOT, np.float32)
    x2_pad[slot_global] = x2[order]
    valid_pad = np.zeros(C * KSLOT, np.float32)
    valid_pad[slot_global] = 1.0

    ident = np.eye(128, dtype=np.float32)
    onesc = np.ones((2, C), ml_dtypes.bfloat16)

    in_maps = []
    for k in range(NCORES):
        lo = k * SLOTS
        hi = lo + SLOTS
        xTk = np.ascontiguousarray(X_pad[lo:hi].T).astype(ml_dtypes.bfloat16)
        x2slot = x2_pad[lo:hi]
        x2hi = x2slot.astype(ml_dtypes.bfloat16)
        x2lo = (x2slot - x2hi.astype(np.float32)).astype(ml_dtypes.bfloat16)
        ones_r = np.ones(SLOTS, ml_dtypes.bfloat16)
        augk = np.ascontiguousarray(np.stack([ones_r, x2hi, x2lo]))
        x2k = np.ascontiguousarray(x2slot.reshape(NT, 128).T)
        vk = np.ascontiguousarray(valid_pad[lo:hi].reshape(NT, 128).T)
        ccore = counts[k * CPC : (k + 1) * CPC].astype(np.float32)
        cnt_col = np.repeat(ccore, 2)
        cntck = np.broadcast_to(cnt_col, (128, NT)).copy()
        acolk = np.broadcast_to(1.0 / (cnt_col - 1.0), (128, NT)).astype(np.float32).copy()
        cnt2k = np.broadcast_to(cnt_col * cnt_col, (128, NT)).astype(np.float32).copy()
        cntrow = ccore.reshape(1, CPC)
        cm1r = (1.0 / (ccore - 1.0)).reshape(1, CPC).astype(np.float32)
        nscl = (-2.0 / ccore).reshape(CPC, 1).astype(np.float32)
        prot = np.zeros((C, C), np.float32)
        src = (k * CPC + np.arange(C)) % C
        prot[src, np.arange(C)] = 1.0
        m = {
            "xT": xTk,
            "aug": augk,
            "onesc": onesc,
            "x2c": x2k,
            "valid": vk,
            "cntc": cntck.astype(np.float32),
            "acol": acolk,
            "cnt2c": cnt2k,
            "cntrow": cntrow.astype(np.float32),
            "cm1r": cm1r,
            "nscl": nscl,
            "prot": prot.astype(ml_dtypes.bfloat16),
            "ident": ident,
        }
        in_maps.append(m)
    return in_maps


def kernel(node_features, community_belong_list):
    global _PROG
    in_maps = _host_prep(node_features, community_belong_list)
    if _PROG is None:
        _PROG = _build_program()
    r = run_bass_kernel_spmd(_PROG, in_maps, list(range(NCORES)))
    tm = sum(float(r.results[k]["out"][0:8, 0].sum()) for k in range(NCORES))
    tn = sum(float(r.results[k]["out"][8:16, 0].sum()) for k in range(NCORES))
    sd = sum(float(r.results[k]["out"][16, 0]) for k in range(NCORES))
    kernel._last_results = r
    out = np.array([tm / N_NODES, tn / N_NODES, sd / (C * D)], dtype=np.float32)
    return out
